# revision 17
# baseline (speedup 1.0000x reference)
import sys

sys.path.insert(0, "/opt/trn_rl_repo")

import numpy as np

B, C, N, OUT = 8, 64, 4096, 128
P = 128
NT = N // P  # 32 row-tiles per sample
NB = 8  # 512-col score chunks per row-tile
CHUNK = N // NB  # 512

# layout of the single packed input 'bigin' [128, BIGF]:
#   cols [0, N):            lhs_aug   (partitions 0..C, row C = ones)
#   cols [N, 2N):           rhs_aug   (partitions 0..C, row C = -0.5*x2)
#   cols [2N, 2N+NT*C):     xts       (x^T tiles: xts[p, t*C+c] = x[c, t*128+p])
#   cols [2N+NT*C, +P):     wt        (W transposed: wt[c, o] = W[o, c])
#   col  [.. +1):           bias
#   cols [.. +P):           diagneg   (-1e30 * I)
#   cols [.. +P):           ident     (I)
XTS_OFF = 2 * N
WT_OFF = XTS_OFF + NT * C
B_OFF = WT_OFF + P
DNEG_OFF = B_OFF + 1
ID_OFF = DNEG_OFF + P
BIGF = ID_OFF + P

_built = None


def _build_kernel():
    import concourse.bass as bass
    import concourse.mybir as mybir
    from concourse import bacc
    from concourse.tile import TileContext

    f32 = mybir.dt.float32
    u32 = mybir.dt.uint32
    AF = mybir.ActivationFunctionType
    MAX = mybir.AluOpType.max

    nc = bacc.Bacc("TRN2")

    big_d = nc.dram_tensor("bigin", [P, BIGF], f32, kind="ExternalInput")
    xt_d = nc.dram_tensor("xt", [N, C], f32, kind="ExternalInput")
    out_d = nc.dram_tensor("out", [OUT, N], f32, kind="ExternalOutput")

    with TileContext(nc) as tc:
        with (
            tc.tile_pool(name="const", bufs=1) as constp,
            tc.tile_pool(name="score", bufs=3) as scorep,
            tc.tile_pool(name="small", bufs=4) as smallp,
            tc.tile_pool(name="ps_s", bufs=4, space="PSUM") as ps_s,
            tc.tile_pool(name="ps_t", bufs=2, space="PSUM") as ps_t,
            tc.tile_pool(name="ps_o", bufs=2, space="PSUM") as ps_o,
        ):
            big = constp.tile([P, BIGF], f32, tag="big")
            # ordered loads: consts first, then a minimal first slice of the
            # score operands so the first matmuls start ASAP, then the rest;
            # xts (stage-b only) last
            nc.sync.dma_start(big[:, WT_OFF:BIGF], big_d[:, WT_OFF:BIGF])
            nc.sync.dma_start(big[0 : C + 1, 0:CHUNK], big_d[0 : C + 1, 0:CHUNK])
            nc.sync.dma_start(
                big[0 : C + 1, N : N + CHUNK], big_d[0 : C + 1, N : N + CHUNK]
            )
            for h in range(4):
                lo, hi = CHUNK + h * 896, CHUNK + (h + 1) * 896
                nc.sync.dma_start(
                    big[0 : C + 1, N + lo : N + hi], big_d[0 : C + 1, N + lo : N + hi]
                )
                nc.sync.dma_start(
                    big[0 : C + 1, lo:hi], big_d[0 : C + 1, lo:hi]
                )
            nc.sync.dma_start(big[:, XTS_OFF:WT_OFF], big_d[:, XTS_OFF:WT_OFF])
            lhs = big[0 : C + 1, 0:N]
            rhs = big[0 : C + 1, N : 2 * N]
            xts = big[:, XTS_OFF:WT_OFF]
            wt = big[:, WT_OFF:B_OFF]
            bsb = big[:, B_OFF : B_OFF + 1]
            dneg = big[:, DNEG_OFF : DNEG_OFF + P]
            ident = big[:, ID_OFF : ID_OFF + P]

            LAG = 3
            Gs = {}

            def stage_a(t):
                # scores for rows [t*128, (t+1)*128) against all N columns
                S = scorep.tile([P, N], f32, tag="S")
                for c8 in range(NB):
                    ps = ps_s.tile([P, CHUNK], f32, tag="ps")
                    nc.tensor.matmul(
                        ps[:],
                        lhsT=lhs[:, t * P : (t + 1) * P],
                        rhs=rhs[:, c8 * CHUNK : (c8 + 1) * CHUNK],
                        start=True,
                        stop=(c8 != t // 4),
                    )
                    if c8 == t // 4:
                        # mask self-distance: accumulate -1e30*I into the
                        # diagonal block of this chunk on the PE
                        nc.tensor.matmul(
                            ps[:, (t % 4) * P : (t % 4 + 1) * P],
                            lhsT=dneg,
                            rhs=ident,
                            start=False,
                            stop=True,
                        )
                    nc.scalar.activation(
                        S[:, c8 * CHUNK : (c8 + 1) * CHUNK], ps[:], AF.Copy
                    )
                # top-8 values + indices per row
                vals = smallp.tile([P, 8], f32, tag="vals")
                idxs = smallp.tile([P, 8], u32, tag="idxs")
                nc.vector.max(out=vals[:], in_=S[:])
                nc.vector.max_index(out=idxs[:], in_max=vals[:], in_values=S[:])
                # gather the 8 neighbor feature rows from xt (DRAM)
                G = smallp.tile([P, 8 * C], f32, tag="G")
                for r in range(8):
                    nc.gpsimd.indirect_dma_start(
                        out=G[:, r * C : (r + 1) * C],
                        out_offset=None,
                        in_=xt_d[:],
                        in_offset=bass.IndirectOffsetOnAxis(
                            ap=idxs[:, r : r + 1], axis=0
                        ),
                    )
                Gs[t] = G

            def stage_b(t):
                G = Gs.pop(t)
                # elementwise max over the 8 gathered rows: one strided reduce
                m3 = smallp.tile([P, C], f32, tag="m3")
                nc.vector.tensor_reduce(
                    out=m3[:],
                    in_=G[:].rearrange("p (r c) -> p c r", r=8),
                    axis=mybir.AxisListType.X,
                    op=MAX,
                )
                # hT = [x_i | relu(gmax - x_i)] in [n, c] layout
                hT = smallp.tile([P, 2 * C], f32, tag="hT")
                nc.scalar.activation(hT[:, 0:C], xts[:, t * C : (t + 1) * C], AF.Copy)
                sub = smallp.tile([P, C], f32, tag="sub")
                nc.vector.tensor_sub(sub[:], m3[:], xts[:, t * C : (t + 1) * C])
                nc.scalar.activation(hT[:, C : 2 * C], sub[:], AF.Relu)
                # transpose hT -> h [c, n]
                hps = ps_t.tile([P, P], f32, tag="hps")
                nc.tensor.transpose(hps[:], hT[:], ident)
                hsb = smallp.tile([P, P], f32, tag="hsb")
                nc.scalar.activation(hsb[:], hps[:], AF.Copy)
                # out[o, n] = relu(W @ h + b)
                ops = ps_o.tile([P, P], f32, tag="ops")
                nc.tensor.matmul(ops[:], lhsT=wt, rhs=hsb[:], start=True, stop=True)
                osb = smallp.tile([P, P], f32, tag="osb")
                nc.scalar.activation(osb[:], ops[:], AF.Relu, bias=bsb)
                nc.sync.dma_start(out_d[:, t * P : (t + 1) * P], osb[:])

            for t in range(NT + LAG):
                if t < NT:
                    stage_a(t)
                if t >= LAG:
                    stage_b(t - LAG)
    nc.compile()
    return nc


def _get_kernel():
    global _built
    if _built is None:
        _built = _build_kernel()
    return _built


def _host_prep(x, W, b):
    con = np.zeros((P, BIGF - WT_OFF), np.float32)
    con[:, 0:P] = W.astype(np.float32).T
    con[:, P] = b.astype(np.float32)
    con[:, P + 1 : 2 * P + 1] = np.eye(P, dtype=np.float32) * np.float32(-1e30)
    con[:, 2 * P + 1 : 3 * P + 1] = np.eye(P, dtype=np.float32)
    in_maps = []
    for i in range(B):
        xb = np.ascontiguousarray(x[i].astype(np.float32))  # [C, N]
        x2 = np.sum(xb * xb, axis=0, dtype=np.float32)  # [N]
        big = np.zeros((P, BIGF), np.float32)
        big[:C, 0:N] = xb
        big[C, 0:N] = 1.0
        big[:C, N : 2 * N] = xb
        big[C, N : 2 * N] = -0.5 * x2
        xT = np.ascontiguousarray(xb.T)  # [N, C]
        big[:, XTS_OFF:WT_OFF] = xT.reshape(NT, P, C).transpose(1, 0, 2).reshape(
            P, NT * C
        )
        big[:, WT_OFF:] = con
        in_maps.append({"bigin": big, "xt": xT})
    return in_maps


def run(x, W, b, **spmd_kwargs):
    from concourse.bass_utils import run_bass_kernel_spmd

    nc = _get_kernel()
    in_maps = _host_prep(x, W, b)
    res = run_bass_kernel_spmd(nc, in_maps, core_ids=list(range(B)), **spmd_kwargs)
    out = np.stack([r["out"] for r in res.results]).astype(np.float32)
    return out, res


def kernel(x, W, b):
    x = np.asarray(x, dtype=np.float32)
    W = np.asarray(W, dtype=np.float32)
    b = np.asarray(b, dtype=np.float32)
    out, _ = run(x, W, b)
    return out


# revision 29
# speedup vs baseline: 1.0452x; 1.0452x over previous
import sys

sys.path.insert(0, "/opt/trn_rl_repo")

import numpy as np

B, C, N, OUT = 8, 64, 4096, 128
P = 128
NT = N // P  # 32 row-tiles per sample
NB = 8  # 512-col score chunks per row-tile
CHUNK = N // NB  # 512

# layout of the single packed input 'bigin' [128, BIGF]:
#   cols [0, N):            lhs_aug   (partitions 0..C, row C = ones)
#   cols [N, 2N):           rhs_aug   (partitions 0..C, row C = -0.5*x2)
#   cols [2N, 2N+NT*C):     xts       (x^T tiles: xts[p, t*C+c] = x[c, t*128+p])
#   cols [2N+NT*C, +P):     wt        (W transposed: wt[c, o] = W[o, c])
#   col  [.. +1):           bias
#   cols [.. +P):           diagneg   (-1e30 * I)
#   cols [.. +P):           ident     (I)
XTS_OFF = 2 * N
WT_OFF = XTS_OFF + NT * C
B_OFF = WT_OFF + P
DNEG_OFF = B_OFF + 1
ID_OFF = DNEG_OFF + P
BIGF = ID_OFF + P

_built = None


def _build_kernel():
    import concourse.bass as bass
    import concourse.mybir as mybir
    from concourse import bacc
    from concourse.tile import TileContext

    f32 = mybir.dt.float32
    u32 = mybir.dt.uint32
    AF = mybir.ActivationFunctionType
    MAX = mybir.AluOpType.max

    nc = bacc.Bacc("TRN2")

    big_d = nc.dram_tensor("bigin", [P, BIGF], f32, kind="ExternalInput")
    xt_d = nc.dram_tensor("xt", [N, C], f32, kind="ExternalInput")
    out_d = nc.dram_tensor("out", [OUT, N], f32, kind="ExternalOutput")

    with TileContext(nc) as tc:
        with (
            tc.tile_pool(name="const", bufs=1) as constp,
            tc.tile_pool(name="score", bufs=2) as scorep,
            tc.tile_pool(name="small", bufs=16) as smallp,
            tc.tile_pool(name="ps_s", bufs=4, space="PSUM") as ps_s,
            tc.tile_pool(name="ps_t", bufs=2, space="PSUM") as ps_t,
            tc.tile_pool(name="ps_o", bufs=2, space="PSUM") as ps_o,
        ):
            big = constp.tile([P, BIGF], f32, tag="big")
            # ordered loads: ident first (gates the PE warmup), then the
            # minimal first slices the first matmuls need, then consts and
            # the rest; xts (stage-b only) last
            nc.sync.dma_start(big[:, ID_OFF : ID_OFF + P], big_d[:, ID_OFF : ID_OFF + P])
            nc.sync.dma_start(big[0 : C + 1, 0:P], big_d[0 : C + 1, 0:P])
            nc.sync.dma_start(
                big[0 : C + 1, N : N + CHUNK], big_d[0 : C + 1, N : N + CHUNK]
            )
            nc.sync.dma_start(big[:, WT_OFF:ID_OFF], big_d[:, WT_OFF:ID_OFF])
            nc.sync.dma_start(big[0 : C + 1, P:CHUNK], big_d[0 : C + 1, P:CHUNK])
            for h in range(4):
                lo, hi = CHUNK + h * 896, CHUNK + (h + 1) * 896
                nc.sync.dma_start(
                    big[0 : C + 1, N + lo : N + hi], big_d[0 : C + 1, N + lo : N + hi]
                )
            for h in range(4):
                lo, hi = CHUNK + h * 896, CHUNK + (h + 1) * 896
                nc.sync.dma_start(big[0 : C + 1, lo:hi], big_d[0 : C + 1, lo:hi])
            nc.sync.dma_start(big[:, XTS_OFF:WT_OFF], big_d[:, XTS_OFF:WT_OFF])
            lhs = big[0 : C + 1, 0:N]
            rhs = big[0 : C + 1, N : 2 * N]
            xts = big[:, XTS_OFF:WT_OFF]
            wt = big[:, WT_OFF:B_OFF]
            bsb = big[:, B_OFF : B_OFF + 1]
            dneg = big[:, DNEG_OFF : DNEG_OFF + P]
            ident = big[:, ID_OFF : ID_OFF + P]

            # warm up the PE (HAM clock gate) while the inputs are still
            # loading: ~4us of dummy matmuls on a memset tile trips the
            # un-throttle window so the first real matmuls run at 2.4GHz.
            # The memset (Pool) is ready ~1.5us before the first DMA lands.
            warm = smallp.tile([P, P], f32, tag="warm")
            nc.gpsimd.memset(warm[:], 0.0)
            for w in range(8):
                wps = ps_t.tile([P, P], f32, tag="hps")
                nc.tensor.matmul(wps[:], lhsT=warm[:], rhs=warm[:], start=True, stop=True)

            LAG = 13
            Gs = {}

            def stage_a(t):
                # scores for rows [t*128, (t+1)*128) against all N columns
                S = scorep.tile([P, N], f32, tag="S")
                for c8 in range(NB):
                    ps = ps_s.tile([P, CHUNK], f32, tag="ps")
                    nc.tensor.matmul(
                        ps[:],
                        lhsT=lhs[:, t * P : (t + 1) * P],
                        rhs=rhs[:, c8 * CHUNK : (c8 + 1) * CHUNK],
                        start=True,
                        stop=(c8 != t // 4),
                    )
                    if c8 == t // 4:
                        # mask self-distance: accumulate -1e30*I into the
                        # diagonal block of this chunk on the PE
                        nc.tensor.matmul(
                            ps[:, (t % 4) * P : (t % 4 + 1) * P],
                            lhsT=dneg,
                            rhs=ident,
                            start=False,
                            stop=True,
                        )
                    nc.scalar.activation(
                        S[:, c8 * CHUNK : (c8 + 1) * CHUNK], ps[:], AF.Copy
                    )
                # top-8 values + indices per row. For the first tiles the
                # value scan is split into quarters that chase the score
                # copies as they land (exact: top8 of unioned top8s), so the
                # DVE starts ~5us earlier; max_index rescans flat either way.
                vals = smallp.tile([P, 8], f32, tag="vals")
                idxs = smallp.tile([P, 8], u32, tag="idxs")
                if t < 2:
                    vq = smallp.tile([P, 32], f32, tag="vq")
                    for q in range(4):
                        nc.vector.max(
                            out=vq[:, q * 8 : (q + 1) * 8],
                            in_=S[:, q * N // 4 : (q + 1) * N // 4],
                        )
                    nc.vector.max(out=vals[:], in_=vq[:])
                else:
                    nc.vector.max(out=vals[:], in_=S[:])
                nc.vector.max_index(out=idxs[:], in_max=vals[:], in_values=S[:])
                # gather the 8 neighbor feature rows from xt (DRAM)
                G = smallp.tile([P, 8 * C], f32, tag="G")
                for r in range(8):
                    nc.gpsimd.indirect_dma_start(
                        out=G[:, r * C : (r + 1) * C],
                        out_offset=None,
                        in_=xt_d[:],
                        in_offset=bass.IndirectOffsetOnAxis(
                            ap=idxs[:, r : r + 1], axis=0
                        ),
                    )
                Gs[t] = G

            def stage_b(t):
                G = Gs.pop(t)
                # elementwise max over the 8 gathered rows: one strided reduce
                m3 = smallp.tile([P, C], f32, tag="m3")
                nc.vector.tensor_reduce(
                    out=m3[:],
                    in_=G[:].rearrange("p (r c) -> p c r", r=8),
                    axis=mybir.AxisListType.X,
                    op=MAX,
                )
                # hT = [x_i | relu(gmax - x_i)] in [n, c] layout
                hT = smallp.tile([P, 2 * C], f32, tag="hT")
                nc.scalar.activation(hT[:, 0:C], xts[:, t * C : (t + 1) * C], AF.Copy)
                sub = smallp.tile([P, C], f32, tag="sub")
                nc.vector.tensor_sub(sub[:], m3[:], xts[:, t * C : (t + 1) * C])
                nc.scalar.activation(hT[:, C : 2 * C], sub[:], AF.Relu)
                # transpose hT -> h [c, n]
                hps = ps_t.tile([P, P], f32, tag="hps")
                nc.tensor.transpose(hps[:], hT[:], ident)
                hsb = smallp.tile([P, P], f32, tag="hsb")
                nc.scalar.activation(hsb[:], hps[:], AF.Copy)
                # out[o, n] = relu(W @ h + b)
                ops = ps_o.tile([P, P], f32, tag="ops")
                nc.tensor.matmul(ops[:], lhsT=wt, rhs=hsb[:], start=True, stop=True)
                osb = smallp.tile([P, P], f32, tag="osb")
                nc.scalar.activation(osb[:], ops[:], AF.Relu, bias=bsb)
                nc.sync.dma_start(out_d[:, t * P : (t + 1) * P], osb[:])

            for t in range(NT + LAG):
                if t < NT:
                    stage_a(t)
                if t >= LAG:
                    stage_b(t - LAG)
    nc.compile()
    return nc


def _get_kernel():
    global _built
    if _built is None:
        _built = _build_kernel()
    return _built


def _host_prep(x, W, b):
    con = np.zeros((P, BIGF - WT_OFF), np.float32)
    con[:, 0:P] = W.astype(np.float32).T
    con[:, P] = b.astype(np.float32)
    con[:, P + 1 : 2 * P + 1] = np.eye(P, dtype=np.float32) * np.float32(-1e30)
    con[:, 2 * P + 1 : 3 * P + 1] = np.eye(P, dtype=np.float32)
    in_maps = []
    for i in range(B):
        xb = np.ascontiguousarray(x[i].astype(np.float32))  # [C, N]
        x2 = np.sum(xb * xb, axis=0, dtype=np.float32)  # [N]
        big = np.zeros((P, BIGF), np.float32)
        big[:C, 0:N] = xb
        big[C, 0:N] = 1.0
        big[:C, N : 2 * N] = xb
        big[C, N : 2 * N] = -0.5 * x2
        xT = np.ascontiguousarray(xb.T)  # [N, C]
        big[:, XTS_OFF:WT_OFF] = xT.reshape(NT, P, C).transpose(1, 0, 2).reshape(
            P, NT * C
        )
        big[:, WT_OFF:] = con
        in_maps.append({"bigin": big, "xt": xT})
    return in_maps


def run(x, W, b, **spmd_kwargs):
    from concourse.bass_utils import run_bass_kernel_spmd

    nc = _get_kernel()
    in_maps = _host_prep(x, W, b)
    res = run_bass_kernel_spmd(nc, in_maps, core_ids=list(range(B)), **spmd_kwargs)
    out = np.stack([r["out"] for r in res.results]).astype(np.float32)
    return out, res


def kernel(x, W, b):
    x = np.asarray(x, dtype=np.float32)
    W = np.asarray(W, dtype=np.float32)
    b = np.asarray(b, dtype=np.float32)
    out, _ = run(x, W, b)
    return out


# revision 30
# speedup vs baseline: 1.0459x; 1.0007x over previous
import sys

sys.path.insert(0, "/opt/trn_rl_repo")

import numpy as np

B, C, N, OUT = 8, 64, 4096, 128
P = 128
NT = N // P  # 32 row-tiles per sample
NB = 8  # 512-col score chunks per row-tile
CHUNK = N // NB  # 512

# layout of the single packed input 'bigin' [128, BIGF]:
#   cols [0, N):            lhs_aug   (partitions 0..C, row C = ones)
#   cols [N, 2N):           rhs_aug   (partitions 0..C, row C = -0.5*x2)
#   cols [2N, 2N+NT*C):     xts       (x^T tiles: xts[p, t*C+c] = x[c, t*128+p])
#   cols [2N+NT*C, +P):     wt        (W transposed: wt[c, o] = W[o, c])
#   col  [.. +1):           bias
#   cols [.. +P):           diagneg   (-1e30 * I)
#   cols [.. +P):           ident     (I)
XTS_OFF = 2 * N
WT_OFF = XTS_OFF + NT * C
B_OFF = WT_OFF + P
DNEG_OFF = B_OFF + 1
ID_OFF = DNEG_OFF + P
BIGF = ID_OFF + P

_built = None


def _build_kernel():
    import concourse.bass as bass
    import concourse.mybir as mybir
    from concourse import bacc
    from concourse.tile import TileContext

    f32 = mybir.dt.float32
    u32 = mybir.dt.uint32
    AF = mybir.ActivationFunctionType
    MAX = mybir.AluOpType.max

    nc = bacc.Bacc("TRN2")

    big_d = nc.dram_tensor("bigin", [P, BIGF], f32, kind="ExternalInput")
    xt_d = nc.dram_tensor("xt", [N, C], f32, kind="ExternalInput")
    out_d = nc.dram_tensor("out", [OUT, N], f32, kind="ExternalOutput")

    with TileContext(nc) as tc:
        with (
            tc.tile_pool(name="const", bufs=1) as constp,
            tc.tile_pool(name="score", bufs=2) as scorep,
            tc.tile_pool(name="small", bufs=16) as smallp,
            tc.tile_pool(name="ps_s", bufs=4, space="PSUM") as ps_s,
            tc.tile_pool(name="ps_t", bufs=2, space="PSUM") as ps_t,
            tc.tile_pool(name="ps_o", bufs=2, space="PSUM") as ps_o,
        ):
            big = constp.tile([P, BIGF], f32, tag="big")
            # ordered loads: ident first (gates the PE warmup), then the
            # minimal first slices the first matmuls need, then consts and
            # the rest; xts (stage-b only) last
            nc.sync.dma_start(big[:, ID_OFF : ID_OFF + P], big_d[:, ID_OFF : ID_OFF + P])
            nc.sync.dma_start(big[0 : C + 1, 0:P], big_d[0 : C + 1, 0:P])
            nc.sync.dma_start(
                big[0 : C + 1, N : N + CHUNK], big_d[0 : C + 1, N : N + CHUNK]
            )
            nc.sync.dma_start(big[:, WT_OFF:ID_OFF], big_d[:, WT_OFF:ID_OFF])
            nc.sync.dma_start(big[0 : C + 1, P:CHUNK], big_d[0 : C + 1, P:CHUNK])
            for h in range(4):
                lo, hi = CHUNK + h * 896, CHUNK + (h + 1) * 896
                nc.sync.dma_start(
                    big[0 : C + 1, N + lo : N + hi], big_d[0 : C + 1, N + lo : N + hi]
                )
            for h in range(4):
                lo, hi = CHUNK + h * 896, CHUNK + (h + 1) * 896
                nc.sync.dma_start(big[0 : C + 1, lo:hi], big_d[0 : C + 1, lo:hi])
            nc.sync.dma_start(big[:, XTS_OFF:WT_OFF], big_d[:, XTS_OFF:WT_OFF])
            lhs = big[0 : C + 1, 0:N]
            rhs = big[0 : C + 1, N : 2 * N]
            xts = big[:, XTS_OFF:WT_OFF]
            wt = big[:, WT_OFF:B_OFF]
            bsb = big[:, B_OFF : B_OFF + 1]
            dneg = big[:, DNEG_OFF : DNEG_OFF + P]
            ident = big[:, ID_OFF : ID_OFF + P]

            # warm up the PE (HAM clock gate) while the inputs are still
            # loading: ~4us of dummy matmuls on a memset tile trips the
            # un-throttle window so the first real matmuls run at 2.4GHz.
            # The memset (Pool) is ready ~1.5us before the first DMA lands.
            warm = smallp.tile([P, P], f32, tag="warm")
            nc.gpsimd.memset(warm[:], 0.0)
            for w in range(8):
                wps = ps_t.tile([P, P], f32, tag="hps")
                nc.tensor.matmul(wps[:], lhsT=warm[:], rhs=warm[:], start=True, stop=True)

            LAG = 13
            Gs = {}

            def stage_a(t):
                # scores for rows [t*128, (t+1)*128) against all N columns
                S = scorep.tile([P, N], f32, tag="S")
                for c8 in range(NB):
                    ps = ps_s.tile([P, CHUNK], f32, tag="ps")
                    nc.tensor.matmul(
                        ps[:],
                        lhsT=lhs[:, t * P : (t + 1) * P],
                        rhs=rhs[:, c8 * CHUNK : (c8 + 1) * CHUNK],
                        start=True,
                        stop=(c8 != t // 4),
                    )
                    if c8 == t // 4:
                        # mask self-distance: accumulate -1e30*I into the
                        # diagonal block of this chunk on the PE
                        nc.tensor.matmul(
                            ps[:, (t % 4) * P : (t % 4 + 1) * P],
                            lhsT=dneg,
                            rhs=ident,
                            start=False,
                            stop=True,
                        )
                    nc.scalar.activation(
                        S[:, c8 * CHUNK : (c8 + 1) * CHUNK], ps[:], AF.Copy
                    )
                # top-8 values + indices per row. For the first tiles the
                # value scan is split into quarters that chase the score
                # copies as they land (exact: top8 of unioned top8s), so the
                # DVE starts ~5us earlier; max_index rescans flat either way.
                vals = smallp.tile([P, 8], f32, tag="vals")
                idxs = smallp.tile([P, 8], u32, tag="idxs")
                if t < 2:
                    vq = smallp.tile([P, 64], f32, tag="vq")
                    for q in range(8):
                        nc.vector.max(
                            out=vq[:, q * 8 : (q + 1) * 8],
                            in_=S[:, q * N // 8 : (q + 1) * N // 8],
                        )
                    nc.vector.max(out=vals[:], in_=vq[:])
                else:
                    nc.vector.max(out=vals[:], in_=S[:])
                nc.vector.max_index(out=idxs[:], in_max=vals[:], in_values=S[:])
                # gather the 8 neighbor feature rows from xt (DRAM)
                G = smallp.tile([P, 8 * C], f32, tag="G")
                for r in range(8):
                    nc.gpsimd.indirect_dma_start(
                        out=G[:, r * C : (r + 1) * C],
                        out_offset=None,
                        in_=xt_d[:],
                        in_offset=bass.IndirectOffsetOnAxis(
                            ap=idxs[:, r : r + 1], axis=0
                        ),
                    )
                Gs[t] = G

            def stage_b(t):
                G = Gs.pop(t)
                # elementwise max over the 8 gathered rows: one strided reduce
                m3 = smallp.tile([P, C], f32, tag="m3")
                nc.vector.tensor_reduce(
                    out=m3[:],
                    in_=G[:].rearrange("p (r c) -> p c r", r=8),
                    axis=mybir.AxisListType.X,
                    op=MAX,
                )
                # hT = [x_i | relu(gmax - x_i)] in [n, c] layout
                hT = smallp.tile([P, 2 * C], f32, tag="hT")
                nc.scalar.activation(hT[:, 0:C], xts[:, t * C : (t + 1) * C], AF.Copy)
                sub = smallp.tile([P, C], f32, tag="sub")
                nc.vector.tensor_sub(sub[:], m3[:], xts[:, t * C : (t + 1) * C])
                nc.scalar.activation(hT[:, C : 2 * C], sub[:], AF.Relu)
                # transpose hT -> h [c, n]
                hps = ps_t.tile([P, P], f32, tag="hps")
                nc.tensor.transpose(hps[:], hT[:], ident)
                hsb = smallp.tile([P, P], f32, tag="hsb")
                nc.scalar.activation(hsb[:], hps[:], AF.Copy)
                # out[o, n] = relu(W @ h + b)
                ops = ps_o.tile([P, P], f32, tag="ops")
                nc.tensor.matmul(ops[:], lhsT=wt, rhs=hsb[:], start=True, stop=True)
                osb = smallp.tile([P, P], f32, tag="osb")
                nc.scalar.activation(osb[:], ops[:], AF.Relu, bias=bsb)
                nc.sync.dma_start(out_d[:, t * P : (t + 1) * P], osb[:])

            for t in range(NT + LAG):
                if t < NT:
                    stage_a(t)
                if t >= LAG:
                    stage_b(t - LAG)
    nc.compile()
    return nc


def _get_kernel():
    global _built
    if _built is None:
        _built = _build_kernel()
    return _built


def _host_prep(x, W, b):
    con = np.zeros((P, BIGF - WT_OFF), np.float32)
    con[:, 0:P] = W.astype(np.float32).T
    con[:, P] = b.astype(np.float32)
    con[:, P + 1 : 2 * P + 1] = np.eye(P, dtype=np.float32) * np.float32(-1e30)
    con[:, 2 * P + 1 : 3 * P + 1] = np.eye(P, dtype=np.float32)
    in_maps = []
    for i in range(B):
        xb = np.ascontiguousarray(x[i].astype(np.float32))  # [C, N]
        x2 = np.sum(xb * xb, axis=0, dtype=np.float32)  # [N]
        big = np.zeros((P, BIGF), np.float32)
        big[:C, 0:N] = xb
        big[C, 0:N] = 1.0
        big[:C, N : 2 * N] = xb
        big[C, N : 2 * N] = -0.5 * x2
        xT = np.ascontiguousarray(xb.T)  # [N, C]
        big[:, XTS_OFF:WT_OFF] = xT.reshape(NT, P, C).transpose(1, 0, 2).reshape(
            P, NT * C
        )
        big[:, WT_OFF:] = con
        in_maps.append({"bigin": big, "xt": xT})
    return in_maps


def run(x, W, b, **spmd_kwargs):
    from concourse.bass_utils import run_bass_kernel_spmd

    nc = _get_kernel()
    in_maps = _host_prep(x, W, b)
    res = run_bass_kernel_spmd(nc, in_maps, core_ids=list(range(B)), **spmd_kwargs)
    out = np.stack([r["out"] for r in res.results]).astype(np.float32)
    return out, res


def kernel(x, W, b):
    x = np.asarray(x, dtype=np.float32)
    W = np.asarray(W, dtype=np.float32)
    b = np.asarray(b, dtype=np.float32)
    out, _ = run(x, W, b)
    return out


# revision 31
# speedup vs baseline: 1.0470x; 1.0010x over previous
import sys

sys.path.insert(0, "/opt/trn_rl_repo")

import numpy as np

B, C, N, OUT = 8, 64, 4096, 128
P = 128
NT = N // P  # 32 row-tiles per sample
NB = 8  # 512-col score chunks per row-tile
CHUNK = N // NB  # 512

# layout of the single packed input 'bigin' [128, BIGF]:
#   cols [0, N):            lhs_aug   (partitions 0..C, row C = ones)
#   cols [N, 2N):           rhs_aug   (partitions 0..C, row C = -0.5*x2)
#   cols [2N, 2N+NT*C):     xts       (x^T tiles: xts[p, t*C+c] = x[c, t*128+p])
#   cols [2N+NT*C, +P):     wt        (W transposed: wt[c, o] = W[o, c])
#   col  [.. +1):           bias
#   cols [.. +P):           diagneg   (-1e30 * I)
#   cols [.. +P):           ident     (I)
XTS_OFF = 2 * N
WT_OFF = XTS_OFF + NT * C
B_OFF = WT_OFF + P
DNEG_OFF = B_OFF + 1
ID_OFF = DNEG_OFF + P
BIGF = ID_OFF + P

_built = None


def _build_kernel():
    import concourse.bass as bass
    import concourse.mybir as mybir
    from concourse import bacc
    from concourse.tile import TileContext

    f32 = mybir.dt.float32
    u32 = mybir.dt.uint32
    AF = mybir.ActivationFunctionType
    MAX = mybir.AluOpType.max

    nc = bacc.Bacc("TRN2")

    big_d = nc.dram_tensor("bigin", [P, BIGF], f32, kind="ExternalInput")
    xt_d = nc.dram_tensor("xt", [N, C], f32, kind="ExternalInput")
    out_d = nc.dram_tensor("out", [OUT, N], f32, kind="ExternalOutput")

    with TileContext(nc) as tc:
        with (
            tc.tile_pool(name="const", bufs=1) as constp,
            tc.tile_pool(name="score", bufs=2) as scorep,
            tc.tile_pool(name="small", bufs=16) as smallp,
            tc.tile_pool(name="ps_s", bufs=4, space="PSUM") as ps_s,
            tc.tile_pool(name="ps_t", bufs=2, space="PSUM") as ps_t,
            tc.tile_pool(name="ps_o", bufs=2, space="PSUM") as ps_o,
        ):
            big = constp.tile([P, BIGF], f32, tag="big")
            # ordered loads: ident first (gates the PE warmup), then the
            # minimal first slices the first matmuls need, then consts and
            # the rest; xts (stage-b only) last
            nc.sync.dma_start(big[:, ID_OFF : ID_OFF + P], big_d[:, ID_OFF : ID_OFF + P])
            nc.sync.dma_start(big[0 : C + 1, 0:P], big_d[0 : C + 1, 0:P])
            nc.sync.dma_start(
                big[0 : C + 1, N : N + CHUNK], big_d[0 : C + 1, N : N + CHUNK]
            )
            nc.sync.dma_start(big[:, WT_OFF:ID_OFF], big_d[:, WT_OFF:ID_OFF])
            nc.sync.dma_start(big[0 : C + 1, P:CHUNK], big_d[0 : C + 1, P:CHUNK])
            rhs_cuts = [CHUNK, 1024, 1536, 2560, 3584, 4096]
            for h in range(len(rhs_cuts) - 1):
                lo, hi = rhs_cuts[h], rhs_cuts[h + 1]
                nc.sync.dma_start(
                    big[0 : C + 1, N + lo : N + hi], big_d[0 : C + 1, N + lo : N + hi]
                )
            for h in range(4):
                lo, hi = CHUNK + h * 896, CHUNK + (h + 1) * 896
                nc.sync.dma_start(big[0 : C + 1, lo:hi], big_d[0 : C + 1, lo:hi])
            nc.sync.dma_start(big[:, XTS_OFF:WT_OFF], big_d[:, XTS_OFF:WT_OFF])
            lhs = big[0 : C + 1, 0:N]
            rhs = big[0 : C + 1, N : 2 * N]
            xts = big[:, XTS_OFF:WT_OFF]
            wt = big[:, WT_OFF:B_OFF]
            bsb = big[:, B_OFF : B_OFF + 1]
            dneg = big[:, DNEG_OFF : DNEG_OFF + P]
            ident = big[:, ID_OFF : ID_OFF + P]

            # warm up the PE (HAM clock gate) while the inputs are still
            # loading: ~4us of dummy matmuls on a memset tile trips the
            # un-throttle window so the first real matmuls run at 2.4GHz.
            # The memset (Pool) is ready ~1.5us before the first DMA lands.
            warm = smallp.tile([P, P], f32, tag="warm")
            nc.gpsimd.memset(warm[:], 0.0)
            for w in range(8):
                wps = ps_t.tile([P, P], f32, tag="hps")
                nc.tensor.matmul(wps[:], lhsT=warm[:], rhs=warm[:], start=True, stop=True)

            LAG = 13
            Gs = {}

            def stage_a(t):
                # scores for rows [t*128, (t+1)*128) against all N columns
                S = scorep.tile([P, N], f32, tag="S")
                for c8 in range(NB):
                    ps = ps_s.tile([P, CHUNK], f32, tag="ps")
                    nc.tensor.matmul(
                        ps[:],
                        lhsT=lhs[:, t * P : (t + 1) * P],
                        rhs=rhs[:, c8 * CHUNK : (c8 + 1) * CHUNK],
                        start=True,
                        stop=(c8 != t // 4),
                    )
                    if c8 == t // 4:
                        # mask self-distance: accumulate -1e30*I into the
                        # diagonal block of this chunk on the PE
                        nc.tensor.matmul(
                            ps[:, (t % 4) * P : (t % 4 + 1) * P],
                            lhsT=dneg,
                            rhs=ident,
                            start=False,
                            stop=True,
                        )
                    nc.scalar.activation(
                        S[:, c8 * CHUNK : (c8 + 1) * CHUNK], ps[:], AF.Copy
                    )
                # top-8 values + indices per row. For the first tiles the
                # value scan is split into quarters that chase the score
                # copies as they land (exact: top8 of unioned top8s), so the
                # DVE starts ~5us earlier; max_index rescans flat either way.
                vals = smallp.tile([P, 8], f32, tag="vals")
                idxs = smallp.tile([P, 8], u32, tag="idxs")
                if t < 2:
                    vq = smallp.tile([P, 64], f32, tag="vq")
                    for q in range(8):
                        nc.vector.max(
                            out=vq[:, q * 8 : (q + 1) * 8],
                            in_=S[:, q * N // 8 : (q + 1) * N // 8],
                        )
                    nc.vector.max(out=vals[:], in_=vq[:])
                else:
                    nc.vector.max(out=vals[:], in_=S[:])
                nc.vector.max_index(out=idxs[:], in_max=vals[:], in_values=S[:])
                # gather the 8 neighbor feature rows from xt (DRAM)
                G = smallp.tile([P, 8 * C], f32, tag="G")
                for r in range(8):
                    nc.gpsimd.indirect_dma_start(
                        out=G[:, r * C : (r + 1) * C],
                        out_offset=None,
                        in_=xt_d[:],
                        in_offset=bass.IndirectOffsetOnAxis(
                            ap=idxs[:, r : r + 1], axis=0
                        ),
                    )
                Gs[t] = G

            def stage_b(t):
                G = Gs.pop(t)
                # elementwise max over the 8 gathered rows: one strided reduce
                m3 = smallp.tile([P, C], f32, tag="m3")
                nc.vector.tensor_reduce(
                    out=m3[:],
                    in_=G[:].rearrange("p (r c) -> p c r", r=8),
                    axis=mybir.AxisListType.X,
                    op=MAX,
                )
                # hT = [x_i | relu(gmax - x_i)] in [n, c] layout
                hT = smallp.tile([P, 2 * C], f32, tag="hT")
                nc.scalar.activation(hT[:, 0:C], xts[:, t * C : (t + 1) * C], AF.Copy)
                sub = smallp.tile([P, C], f32, tag="sub")
                nc.vector.tensor_sub(sub[:], m3[:], xts[:, t * C : (t + 1) * C])
                nc.scalar.activation(hT[:, C : 2 * C], sub[:], AF.Relu)
                # transpose hT -> h [c, n]
                hps = ps_t.tile([P, P], f32, tag="hps")
                nc.tensor.transpose(hps[:], hT[:], ident)
                hsb = smallp.tile([P, P], f32, tag="hsb")
                nc.scalar.activation(hsb[:], hps[:], AF.Copy)
                # out[o, n] = relu(W @ h + b)
                ops = ps_o.tile([P, P], f32, tag="ops")
                nc.tensor.matmul(ops[:], lhsT=wt, rhs=hsb[:], start=True, stop=True)
                osb = smallp.tile([P, P], f32, tag="osb")
                nc.scalar.activation(osb[:], ops[:], AF.Relu, bias=bsb)
                nc.sync.dma_start(out_d[:, t * P : (t + 1) * P], osb[:])

            for t in range(NT + LAG):
                if t < NT:
                    stage_a(t)
                if t >= LAG:
                    stage_b(t - LAG)
    nc.compile()
    return nc


def _get_kernel():
    global _built
    if _built is None:
        _built = _build_kernel()
    return _built


def _host_prep(x, W, b):
    con = np.zeros((P, BIGF - WT_OFF), np.float32)
    con[:, 0:P] = W.astype(np.float32).T
    con[:, P] = b.astype(np.float32)
    con[:, P + 1 : 2 * P + 1] = np.eye(P, dtype=np.float32) * np.float32(-1e30)
    con[:, 2 * P + 1 : 3 * P + 1] = np.eye(P, dtype=np.float32)
    in_maps = []
    for i in range(B):
        xb = np.ascontiguousarray(x[i].astype(np.float32))  # [C, N]
        x2 = np.sum(xb * xb, axis=0, dtype=np.float32)  # [N]
        big = np.zeros((P, BIGF), np.float32)
        big[:C, 0:N] = xb
        big[C, 0:N] = 1.0
        big[:C, N : 2 * N] = xb
        big[C, N : 2 * N] = -0.5 * x2
        xT = np.ascontiguousarray(xb.T)  # [N, C]
        big[:, XTS_OFF:WT_OFF] = xT.reshape(NT, P, C).transpose(1, 0, 2).reshape(
            P, NT * C
        )
        big[:, WT_OFF:] = con
        in_maps.append({"bigin": big, "xt": xT})
    return in_maps


def run(x, W, b, **spmd_kwargs):
    from concourse.bass_utils import run_bass_kernel_spmd

    nc = _get_kernel()
    in_maps = _host_prep(x, W, b)
    res = run_bass_kernel_spmd(nc, in_maps, core_ids=list(range(B)), **spmd_kwargs)
    out = np.stack([r["out"] for r in res.results]).astype(np.float32)
    return out, res


def kernel(x, W, b):
    x = np.asarray(x, dtype=np.float32)
    W = np.asarray(W, dtype=np.float32)
    b = np.asarray(b, dtype=np.float32)
    out, _ = run(x, W, b)
    return out


# revision 32
# speedup vs baseline: 1.0471x; 1.0001x over previous
import sys

sys.path.insert(0, "/opt/trn_rl_repo")

import numpy as np

B, C, N, OUT = 8, 64, 4096, 128
P = 128
NT = N // P  # 32 row-tiles per sample
NB = 8  # 512-col score chunks per row-tile
CHUNK = N // NB  # 512

# layout of the single packed input 'bigin' [128, BIGF]:
#   cols [0, N):            lhs_aug   (partitions 0..C, row C = ones)
#   cols [N, 2N):           rhs_aug   (partitions 0..C, row C = -0.5*x2)
#   cols [2N, 2N+NT*C):     xts       (x^T tiles: xts[p, t*C+c] = x[c, t*128+p])
#   cols [2N+NT*C, +P):     wt        (W transposed: wt[c, o] = W[o, c])
#   col  [.. +1):           bias
#   cols [.. +P):           diagneg   (-1e30 * I)
#   cols [.. +P):           ident     (I)
XTS_OFF = 2 * N
WT_OFF = XTS_OFF + NT * C
B_OFF = WT_OFF + P
DNEG_OFF = B_OFF + 1
ID_OFF = DNEG_OFF + P
BIGF = ID_OFF + P

_built = None


def _build_kernel():
    import concourse.bass as bass
    import concourse.mybir as mybir
    from concourse import bacc
    from concourse.tile import TileContext

    f32 = mybir.dt.float32
    u32 = mybir.dt.uint32
    AF = mybir.ActivationFunctionType
    MAX = mybir.AluOpType.max

    nc = bacc.Bacc("TRN2")

    big_d = nc.dram_tensor("bigin", [P, BIGF], f32, kind="ExternalInput")
    xt_d = nc.dram_tensor("xt", [N, C], f32, kind="ExternalInput")
    out_d = nc.dram_tensor("out", [OUT, N], f32, kind="ExternalOutput")

    with TileContext(nc) as tc:
        with (
            tc.tile_pool(name="const", bufs=1) as constp,
            tc.tile_pool(name="score", bufs=2) as scorep,
            tc.tile_pool(name="small", bufs=16) as smallp,
            tc.tile_pool(name="ps_s", bufs=4, space="PSUM") as ps_s,
            tc.tile_pool(name="ps_t", bufs=2, space="PSUM") as ps_t,
            tc.tile_pool(name="ps_o", bufs=2, space="PSUM") as ps_o,
        ):
            big = constp.tile([P, BIGF], f32, tag="big")
            # ordered loads: ident first (gates the PE warmup), then the
            # minimal first slices the first matmuls need, then consts and
            # the rest; xts (stage-b only) last
            nc.sync.dma_start(big[:, ID_OFF : ID_OFF + P], big_d[:, ID_OFF : ID_OFF + P])
            nc.sync.dma_start(big[0 : C + 1, 0:P], big_d[0 : C + 1, 0:P])
            nc.sync.dma_start(
                big[0 : C + 1, N : N + CHUNK], big_d[0 : C + 1, N : N + CHUNK]
            )
            nc.sync.dma_start(big[:, WT_OFF:ID_OFF], big_d[:, WT_OFF:ID_OFF])
            rhs_cuts = [CHUNK, 1024, 1536, 2560, 3584, 4096]
            for h in range(len(rhs_cuts) - 1):
                lo, hi = rhs_cuts[h], rhs_cuts[h + 1]
                nc.sync.dma_start(
                    big[0 : C + 1, N + lo : N + hi], big_d[0 : C + 1, N + lo : N + hi]
                )
                if h == 1:
                    nc.sync.dma_start(
                        big[0 : C + 1, P:CHUNK], big_d[0 : C + 1, P:CHUNK]
                    )
            for h in range(4):
                lo, hi = CHUNK + h * 896, CHUNK + (h + 1) * 896
                nc.sync.dma_start(big[0 : C + 1, lo:hi], big_d[0 : C + 1, lo:hi])
            nc.sync.dma_start(big[:, XTS_OFF:WT_OFF], big_d[:, XTS_OFF:WT_OFF])
            lhs = big[0 : C + 1, 0:N]
            rhs = big[0 : C + 1, N : 2 * N]
            xts = big[:, XTS_OFF:WT_OFF]
            wt = big[:, WT_OFF:B_OFF]
            bsb = big[:, B_OFF : B_OFF + 1]
            dneg = big[:, DNEG_OFF : DNEG_OFF + P]
            ident = big[:, ID_OFF : ID_OFF + P]

            # warm up the PE (HAM clock gate) while the inputs are still
            # loading: ~4us of dummy matmuls on a memset tile trips the
            # un-throttle window so the first real matmuls run at 2.4GHz.
            # The memset (Pool) is ready ~1.5us before the first DMA lands.
            warm = smallp.tile([P, P], f32, tag="warm")
            nc.gpsimd.memset(warm[:], 0.0)
            for w in range(8):
                wps = ps_t.tile([P, P], f32, tag="hps")
                nc.tensor.matmul(wps[:], lhsT=warm[:], rhs=warm[:], start=True, stop=True)

            LAG = 13
            Gs = {}

            def stage_a(t):
                # scores for rows [t*128, (t+1)*128) against all N columns
                S = scorep.tile([P, N], f32, tag="S")
                for c8 in range(NB):
                    ps = ps_s.tile([P, CHUNK], f32, tag="ps")
                    nc.tensor.matmul(
                        ps[:],
                        lhsT=lhs[:, t * P : (t + 1) * P],
                        rhs=rhs[:, c8 * CHUNK : (c8 + 1) * CHUNK],
                        start=True,
                        stop=(c8 != t // 4),
                    )
                    if c8 == t // 4:
                        # mask self-distance: accumulate -1e30*I into the
                        # diagonal block of this chunk on the PE
                        nc.tensor.matmul(
                            ps[:, (t % 4) * P : (t % 4 + 1) * P],
                            lhsT=dneg,
                            rhs=ident,
                            start=False,
                            stop=True,
                        )
                    nc.scalar.activation(
                        S[:, c8 * CHUNK : (c8 + 1) * CHUNK], ps[:], AF.Copy
                    )
                # top-8 values + indices per row. For the first tiles the
                # value scan is split into quarters that chase the score
                # copies as they land (exact: top8 of unioned top8s), so the
                # DVE starts ~5us earlier; max_index rescans flat either way.
                vals = smallp.tile([P, 8], f32, tag="vals")
                idxs = smallp.tile([P, 8], u32, tag="idxs")
                if t < 2:
                    vq = smallp.tile([P, 64], f32, tag="vq")
                    for q in range(8):
                        nc.vector.max(
                            out=vq[:, q * 8 : (q + 1) * 8],
                            in_=S[:, q * N // 8 : (q + 1) * N // 8],
                        )
                    nc.vector.max(out=vals[:], in_=vq[:])
                else:
                    nc.vector.max(out=vals[:], in_=S[:])
                nc.vector.max_index(out=idxs[:], in_max=vals[:], in_values=S[:])
                # gather the 8 neighbor feature rows from xt (DRAM)
                G = smallp.tile([P, 8 * C], f32, tag="G")
                for r in range(8):
                    nc.gpsimd.indirect_dma_start(
                        out=G[:, r * C : (r + 1) * C],
                        out_offset=None,
                        in_=xt_d[:],
                        in_offset=bass.IndirectOffsetOnAxis(
                            ap=idxs[:, r : r + 1], axis=0
                        ),
                    )
                Gs[t] = G

            def stage_b(t):
                G = Gs.pop(t)
                # elementwise max over the 8 gathered rows: one strided reduce
                m3 = smallp.tile([P, C], f32, tag="m3")
                nc.vector.tensor_reduce(
                    out=m3[:],
                    in_=G[:].rearrange("p (r c) -> p c r", r=8),
                    axis=mybir.AxisListType.X,
                    op=MAX,
                )
                # hT = [x_i | relu(gmax - x_i)] in [n, c] layout
                hT = smallp.tile([P, 2 * C], f32, tag="hT")
                nc.scalar.activation(hT[:, 0:C], xts[:, t * C : (t + 1) * C], AF.Copy)
                sub = smallp.tile([P, C], f32, tag="sub")
                nc.vector.tensor_sub(sub[:], m3[:], xts[:, t * C : (t + 1) * C])
                nc.scalar.activation(hT[:, C : 2 * C], sub[:], AF.Relu)
                # transpose hT -> h [c, n]
                hps = ps_t.tile([P, P], f32, tag="hps")
                nc.tensor.transpose(hps[:], hT[:], ident)
                hsb = smallp.tile([P, P], f32, tag="hsb")
                nc.scalar.activation(hsb[:], hps[:], AF.Copy)
                # out[o, n] = relu(W @ h + b)
                ops = ps_o.tile([P, P], f32, tag="ops")
                nc.tensor.matmul(ops[:], lhsT=wt, rhs=hsb[:], start=True, stop=True)
                osb = smallp.tile([P, P], f32, tag="osb")
                nc.scalar.activation(osb[:], ops[:], AF.Relu, bias=bsb)
                nc.sync.dma_start(out_d[:, t * P : (t + 1) * P], osb[:])

            for t in range(NT + LAG):
                if t < NT:
                    stage_a(t)
                if t >= LAG:
                    stage_b(t - LAG)
    nc.compile()
    return nc


def _get_kernel():
    global _built
    if _built is None:
        _built = _build_kernel()
    return _built


def _host_prep(x, W, b):
    con = np.zeros((P, BIGF - WT_OFF), np.float32)
    con[:, 0:P] = W.astype(np.float32).T
    con[:, P] = b.astype(np.float32)
    con[:, P + 1 : 2 * P + 1] = np.eye(P, dtype=np.float32) * np.float32(-1e30)
    con[:, 2 * P + 1 : 3 * P + 1] = np.eye(P, dtype=np.float32)
    in_maps = []
    for i in range(B):
        xb = np.ascontiguousarray(x[i].astype(np.float32))  # [C, N]
        x2 = np.sum(xb * xb, axis=0, dtype=np.float32)  # [N]
        big = np.zeros((P, BIGF), np.float32)
        big[:C, 0:N] = xb
        big[C, 0:N] = 1.0
        big[:C, N : 2 * N] = xb
        big[C, N : 2 * N] = -0.5 * x2
        xT = np.ascontiguousarray(xb.T)  # [N, C]
        big[:, XTS_OFF:WT_OFF] = xT.reshape(NT, P, C).transpose(1, 0, 2).reshape(
            P, NT * C
        )
        big[:, WT_OFF:] = con
        in_maps.append({"bigin": big, "xt": xT})
    return in_maps


def run(x, W, b, **spmd_kwargs):
    from concourse.bass_utils import run_bass_kernel_spmd

    nc = _get_kernel()
    in_maps = _host_prep(x, W, b)
    res = run_bass_kernel_spmd(nc, in_maps, core_ids=list(range(B)), **spmd_kwargs)
    out = np.stack([r["out"] for r in res.results]).astype(np.float32)
    return out, res


def kernel(x, W, b):
    x = np.asarray(x, dtype=np.float32)
    W = np.asarray(W, dtype=np.float32)
    b = np.asarray(b, dtype=np.float32)
    out, _ = run(x, W, b)
    return out
